# revision 1
# baseline (speedup 1.0000x reference)
"""BalanceCrossEntropyLoss on 8 Trainium2 NeuronCores.

Problem shapes (hardcoded): pred (16,1,1024,1024) f32, gt (16,1,1024,1024) f32,
mask (16,1024,1024) f32.  Output: scalar f32.

Strategy
--------
Data-parallel over the flattened 16M elements: each of the 8 cores gets a
contiguous [128, 16384] f32 shard of pred and gt.

For binary gt and all-ones mask (the spec's fill types), the reference's
hard-negative top-k selects *all* negatives whenever #neg <= floor(3*#pos)
(every negative-pixel loss is > 0 and every other entry of the flattened
negative-loss map is exactly 0), so

    negative_sum = sum(negative_loss)         (no sort/top-k needed)
    balance_loss = (sum(pos_loss) + negative_sum) / (#pos + #neg + eps)

Device kernel (per core) — exploits f_neg(p) = f_pos(1-p) symmetry:
    s  = 2g-1 in bf16 (exact for binary g; made partly on ScalarE Copy,
         partly on VectorE affine_mul_reduce to balance engine load;
         both accum_outs give sum(s) -> #pos for free)
    v0 = (p-1/2)*s = v-1/2 with v = p*g + (1-p)*(1-g)   (one VectorE op)
    M  = ln(v+1e-7) * exp(-v)    (2 ScalarE activations reading v0 with
                                  scale/bias folding + 1 fused VectorE
                                  multiply-reduce -> sum(M))
    M equals -pos_loss at positives and -neg_loss at negatives, so
    sum(pos_loss) = -sum(M*g)  and  sum(neg_loss) = -(sum(M) - sum(M*g)).
    sum(M*s) comes from the TensorEngine: per 128-column chunk,
    psum[m,n] += sum_k s[k,m]*M[k,n] accumulated over all chunks — the
    psum diagonal holds chunkwise partials of M*s; sum(M*g)=(it+sum(M))/2.
    The Ln bias is 1e-7 (not the reference's 1e-37): it guards the fp32
    rounding of v0 near 0 (can undershoot by ~6e-8) and perturbs the sum
    by ~1e-5 relative, far inside the accuracy budget.

Host preconditions (mask all-ones, gt binary, #neg <= floor(3*#pos)) are
checked; any violation falls back to an exact numpy implementation of the
reference (including the true top-k).
"""

import sys

sys.path.insert(0, "/opt/trn_rl_repo")

import numpy as np

N_CORES = 8
P = 128
FREE = 16384          # per-core free dim: 16M / 8 cores / 128 partitions
F = 2048              # tile free dim
NT = FREE // F
TOTAL = 16 * 1024 * 1024
LOG_EPS = 1e-37
LN_EPS = 1e-7         # device Ln bias; guards fp32 cancellation in v (see doc)
NEGATIVE_RATIO = 3.0
EPS = 1e-6

_NC_CACHE = {}


def _patch_act_tables():
    """Restrict Ln/Exp to the combined 'natural_log_exp_and_others' table so
    the act-table-load pass emits one hoisted load instead of per-tile
    switches.  Only affects the copy handed to insert_act_table_loads; the
    table ids still index the compiler's own act_info.json."""
    import concourse.bacc as bacc_mod
    import concourse.mybir as mybir
    from concourse.hw_specs import get_activation_tables as _real

    if getattr(bacc_mod, "_act_tables_patched", False):
        return

    AF = mybir.ActivationFunctionType

    def _combined(arch):
        out = {}
        for name, funcs in _real(arch).items():
            if name == "natural_log_exp_and_others":
                out[name] = set(funcs)
            else:
                out[name] = set(funcs) - {AF.Ln, AF.Exp}
        return out

    bacc_mod.get_activation_tables = _combined
    bacc_mod._act_tables_patched = True


def _build_nc(free=FREE, tile_f=F, debug=False, BUFS=(5, 4, 4, 3)):
    import concourse.bacc as bacc
    import concourse.mybir as mybir
    from concourse.tile import TileContext

    f32 = mybir.dt.float32
    bf16 = mybir.dt.bfloat16
    AF = mybir.ActivationFunctionType
    NT = free // tile_f
    F = tile_f
    DC = 128              # diag-trick chunk width
    NCHUNK = F // DC

    _patch_act_tables()
    nc = bacc.Bacc(None, target_bir_lowering=False, debug=debug)
    pred = nc.declare_dram_parameter("pred", [P, free], f32, isOutput=False)
    gt = nc.declare_dram_parameter("gt", [P, free], f32, isOutput=False)
    # acc col groups of 8 per tile i: 0=sum(M), 1/2=scratch, 3/4=sum(s) parts
    acc_out = nc.declare_dram_parameter("acc", [P, 5 * NT], f32, isOutput=True)
    # (one [P,5] accum tile per input tile; all shipped in one DMA at the end)
    mg_out = nc.declare_dram_parameter("mg", [DC, DC], f32, isOutput=True)
    # split point for s production: ScalarE Copy makes [0:FA), VectorE amr
    # makes [FA:F) — balances real work between the two engines
    import os as _os
    FA = int(_os.environ.get("FA_CHUNKS", str(5 * (F // DC) // 8))) * DC
    FB = F - FA

    with TileContext(nc) as tc:
        with (
            tc.tile_pool(name="const", bufs=1) as cpool,
            tc.tile_pool(name="io", bufs=BUFS[0]) as io,
            tc.tile_pool(name="spool", bufs=BUFS[1]) as spool,
            tc.tile_pool(name="work", bufs=BUFS[2]) as work,
            tc.tile_pool(name="mpool", bufs=BUFS[3]) as mpool,
            tc.tile_pool(name="psum", bufs=1, space="PSUM") as pp,
        ):
            ps = pp.tile([DC, DC], f32)

            def const_ap(val, tag):
                t = cpool.tile([P, 1], f32, tag=tag)
                nc.vector.memset(t[:], val)
                return t

            c_half_eps = const_ap(0.5 + LN_EPS, "c_half_eps")
            c_neghalf = const_ap(-0.5, "c_neghalf")
            ones = cpool.tile([P, FB], bf16)
            nc.vector.memset(ones[:], 1.0)

            # Software-pipelined emission: engines execute in emission order,
            # so stage the per-tile chain sa/sb -> v0 -> lpv/env -> M across
            # four steps; each engine's step-k ops depend only on step-(k-1)
            # outputs and no engine ever waits mid-step on another.
            pt, gt_t, sat, sbt, v0t, lpvt, envt, Mt = ({} for _ in range(8))
            acct = {}
            mm_idx = 0

            def emit_dma(i):
                sl = slice(i * F, (i + 1) * F)
                pt[i] = io.tile([P, F], f32, tag="p", name="p_t")
                gt_t[i] = io.tile([P, F], f32, tag="g", name="g_t")
                nc.sync.dma_start(out=pt[i][:], in_=pred[:, sl])
                nc.sync.dma_start(out=gt_t[i][:], in_=gt[:, sl])

            def emit_s(i):
                # s = 2g-1 in bf16 (exact for binary g); accum -> sum(s);
                # production split ScalarE/VectorE to balance load
                sat[i] = spool.tile([P, FA], bf16, tag="sa", name="sa_t")
                sbt[i] = spool.tile([P, FB], bf16, tag="sb", name="sb_t")
                acct[i] = cpool.tile([P, 5], f32, tag=f"acc{i}", name="acc_t")
                nc.scalar.activation(sat[i][:], gt_t[i][:, :FA], AF.Copy,
                                     bias=-1.0, scale=2.0,
                                     accum_out=acct[i][:, 3:4])
                nc.vector.affine_mul_reduce(
                    out=sbt[i][:], in0=gt_t[i][:, FA:], in1=ones[:], scale=2.0,
                    bias=-1.0, accum_out=acct[i][:, 4:5])

            def emit_v0(i):
                # v0 = (p-1/2)*s = v-1/2 where v = p*g + (1-p)*(1-g)
                v0t[i] = work.tile([P, F], f32, tag="v0", name="v0_t")
                nc.vector.affine_mul_reduce(
                    out=v0t[i][:, :FA], in0=pt[i][:, :FA], in1=sat[i][:],
                    scale=1.0, bias=-0.5, accum_out=acct[i][:, 1:2])
                nc.vector.affine_mul_reduce(
                    out=v0t[i][:, FA:], in0=pt[i][:, FA:], in1=sbt[i][:],
                    scale=1.0, bias=-0.5, accum_out=acct[i][:, 2:3])

            def emit_lnexp(i):
                # lpv = ln(v + 1e-7) = ln(v0 + 1/2 + 1e-7), env = exp(-v)
                lpvt[i] = work.tile([P, F], bf16, tag="lpv", name="lpv_t")
                envt[i] = work.tile([P, F], bf16, tag="env", name="env_t")
                nc.scalar.activation(lpvt[i][:], v0t[i][:], AF.Ln,
                                     bias=c_half_eps[:], scale=1.0)
                nc.scalar.activation(envt[i][:], v0t[i][:], AF.Exp,
                                     bias=c_neghalf[:], scale=-1.0)

            def emit_m_mm(i):
                nonlocal mm_idx
                # M = lpv*env ; accum -> sum(M); then the psum diag matmuls:
                # psum[m,n] += sum_k s[k,m]*M[k,n]
                Mt[i] = mpool.tile([P, F], bf16, tag="M", name="m_t")
                nc.vector.affine_mul_reduce(
                    out=Mt[i][:], in0=lpvt[i][:], in1=envt[i][:],
                    scale=1.0, bias=0.0, accum_out=acct[i][:, 0:1])
                for c in range(NCHUNK):
                    cs = slice(c * DC, (c + 1) * DC)
                    s_chunk = (sat[i][:, cs] if (c + 1) * DC <= FA
                               else sbt[i][:, c * DC - FA : (c + 1) * DC - FA])
                    nc.tensor.matmul(
                        ps[:, :], s_chunk, Mt[i][:, cs],
                        start=(mm_idx == 0),
                        stop=(mm_idx == NT * NCHUNK - 1))
                    mm_idx += 1

            for k in range(NT):
                emit_dma(k)
                emit_s(k)
                emit_v0(k)
                emit_lnexp(k)
                emit_m_mm(k)

            mg_t = cpool.tile([DC, DC], f32)
            nc.vector.tensor_copy(out=mg_t[:], in_=ps[:, :])
            for i in range(NT):
                nc.sync.dma_start(out=acc_out[:, 5 * i : 5 * i + 5],
                                  in_=acct[i][:])
            nc.sync.dma_start(out=mg_out[:, :], in_=mg_t[:])

    nc.finalize()
    return nc


def _get_nc():
    if "nc" not in _NC_CACHE:
        _NC_CACHE["nc"] = _build_nc()
    return _NC_CACHE["nc"]


def _device_sums(pred32, gt32, trace=False, tmpdir=None):
    """pred32/gt32: (8,128,16384) f32. Returns (SM, SMg, GS, results).

    Device reports SM = sum(M), S2 = sum(M*s), G2 = sum(s) with s = 2g-1;
    converts to SMg = (S2+SM)/2 and GS = (G2+TOTAL)/2 here."""
    from concourse.bass_utils import run_bass_kernel_spmd

    nc = _get_nc()
    in_maps = [{"pred": pred32[c], "gt": gt32[c]} for c in range(N_CORES)]
    res = run_bass_kernel_spmd(
        nc, in_maps, core_ids=list(range(N_CORES)), trace=trace, tmpdir=tmpdir)
    SM = S2 = G2 = 0.0
    for c in range(N_CORES):
        a = res.results[c]["acc"].astype(np.float64)
        SM += a[:, 0::5].sum()
        G2 += a[:, 3::5].sum() + a[:, 4::5].sum()
        S2 += np.diagonal(res.results[c]["mg"].astype(np.float64)).sum()
    SMg = 0.5 * (S2 + SM)
    GS = 0.5 * (G2 + float(TOTAL))
    return SM, SMg, GS, res


def _fallback(pred, gt, mask):
    """Exact numpy mirror of the reference (handles arbitrary inputs)."""
    p = pred[:, 0].astype(np.float64)
    g = gt[:, 0].astype(np.float64)
    m = mask.astype(np.float64)
    positive = g * m
    negative = (1.0 - g) * m
    pos_cnt = positive.sum()
    neg_cnt = min(negative.sum(), np.floor(pos_cnt * NEGATIVE_RATIO))
    loss = ((g - 1.0) * np.log(1.0 - p + LOG_EPS) / np.exp(1.0 - p)
            - g * np.log(p + LOG_EPS) / np.exp(p))
    pos_loss = (loss * positive).sum()
    flat_neg = (loss * negative).ravel()
    k = int(np.ceil(neg_cnt - 1e-12)) if neg_cnt > 0 else 0
    if k >= flat_neg.size:
        neg_sum = flat_neg.sum()
    elif k > 0:
        neg_sum = np.partition(flat_neg, flat_neg.size - k)[flat_neg.size - k:].sum()
    else:
        neg_sum = 0.0
    return np.float32((pos_loss + neg_sum) / (pos_cnt + neg_cnt + EPS))


def kernel(pred, gt, mask):
    pred = np.asarray(pred)
    gt = np.asarray(gt)
    mask = np.asarray(mask)
    if not (mask == 1.0).all() or not ((gt == 0.0) | (gt == 1.0)).all():
        return _fallback(pred, gt, mask)

    pr = np.ascontiguousarray(pred, dtype=np.float32).reshape(N_CORES, P, FREE)
    gr = np.ascontiguousarray(gt, dtype=np.float32).reshape(N_CORES, P, FREE)
    SM, SMg, GS, _ = _device_sums(pr, gr)

    pos_cnt = GS
    neg_raw = float(TOTAL) - GS
    neg_count = min(neg_raw, float(np.floor(np.float32(pos_cnt) * np.float32(NEGATIVE_RATIO))))
    if neg_raw > neg_count + 0.5:
        # top-k actually bites; take the exact path
        return _fallback(pred, gt, mask)

    pos_loss = -SMg
    neg_sum = -(SM - SMg)
    return np.float32((pos_loss + neg_sum) / (pos_cnt + neg_count + EPS))



# revision 4
# speedup vs baseline: 1.7027x; 1.7027x over previous
"""BalanceCrossEntropyLoss on 8 Trainium2 NeuronCores.

Problem shapes (hardcoded): pred (16,1,1024,1024) f32, gt (16,1,1024,1024) f32,
mask (16,1024,1024) f32.  Output: scalar f32.

Math
----
With binary gt and an all-ones mask (verified on host; exact fallback
otherwise), every flattened negative-loss entry at a negative pixel is > 0
and every other entry is exactly 0, so whenever #neg <= floor(3*#pos) the
reference's hard-negative top-k selects *all* negatives and

    numerator = sum(positive_loss) + negative_sum = sum_i loss_i = -SM,
    SM = sum_i ln(v_i + eps)*exp(-v_i),   v_i = p_i if g_i=1 else 1-p_i,
    balance_loss = -SM / (#pos + #neg + 1e-6).

So the device only needs the single reduction SM; the counts come from the
host-side validation pass that is required anyway.

Sharding strategy
-----------------
Shard by class: the host routes each pred value into a "positive" or
"negative" column block (a permutation / bucketing of the input driven by
the binary gt mask -- gt itself never needs to be uploaded) and pads the
blocks to a fixed per-core size with neutral values (p=1 for the positive
block, p=0 for the negative block; each contributes ~4e-8 to SM).  Values
are shipped as fp16 (rel. rounding 2^-11; validated ~1e-3 relative error on
the final scalar vs the 2e-2 budget).

Device kernel (per core, 2 blocks x 3 tiles of [128, 2752] fp16):
    lpv  = Ln(s*p + b)            ScalarE, s=+1,b=eps (pos) / s=-1,b=1+eps (neg)
    env1 = (c2*p + c1)*p          VectorE affine_mul_reduce (2x fp16 mode)
    M    = (env1 + c0)*lpv        VectorE amr, accum_out -> per-partition SM
with (c0,c1,c2) a bias-corrected least-squares quadratic for exp(-t) on
[0,1] (max rel err 1.4e-2, zero mean error against the ln weight; the
negative block uses the mirrored coefficients for exp(t-1)).  No gt tensor,
no matmuls: 4.1MB HBM in per core, 2 DVE ops + 1 Act op per element.
"""

import os
import sys

sys.path.insert(0, "/opt/trn_rl_repo")

import numpy as np

N_CORES = 8
P = 128
F = 2752                  # tile free dim
NT_BLK = 3                # tiles per class block
NT = 2 * NT_BLK           # 6 tiles per core
KBLK = NT_BLK * F         # 8256 columns per class block per core
CAP_BLK = N_CORES * P * KBLK   # 8454144 element capacity per class
TOTAL = 16 * 1024 * 1024
LN_EPS = 1e-7             # device Ln bias; baseline-validated vs the 1e-37 ref
NEGATIVE_RATIO = 3.0
EPS = 1e-6

# bias-corrected LS quadratic for exp(-t), t in [0,1]  (see module docstring)
C0, C1, C2 = 0.99493479, -0.93054858, 0.30871856
# mirrored coefficients: exp(p-1) = N0 + N1*p + N2*p^2
N0, N1, N2 = C0 + C1 + C2, -C1 - 2.0 * C2, C2

_NC_CACHE = {}


def _patch_act_tables():
    """Restrict Ln/Exp to the combined 'natural_log_exp_and_others' table so
    the act-table-load pass emits one hoisted load instead of per-tile
    switches."""
    import concourse.bacc as bacc_mod
    import concourse.mybir as mybir
    from concourse.hw_specs import get_activation_tables as _real

    if getattr(bacc_mod, "_act_tables_patched", False):
        return

    AF = mybir.ActivationFunctionType

    def _combined(arch):
        out = {}
        for name, funcs in _real(arch).items():
            if name == "natural_log_exp_and_others":
                out[name] = set(funcs)
            else:
                out[name] = set(funcs) - {AF.Ln, AF.Exp}
        return out

    bacc_mod.get_activation_tables = _combined
    bacc_mod._act_tables_patched = True


def _build_nc(debug=False):
    import concourse.bacc as bacc
    import concourse.mybir as mybir
    from concourse.tile import TileContext

    f32 = mybir.dt.float32
    f16 = mybir.dt.float16
    AF = mybir.ActivationFunctionType

    _patch_act_tables()
    nc = bacc.Bacc(None, target_bir_lowering=False, debug=debug)
    # tile-major layout: tile k = rows [128k, 128k+128)
    pk = nc.declare_dram_parameter("pk", [NT * P, F], f16, isOutput=False)
    acc_out = nc.declare_dram_parameter("acc", [P, NT], f32, isOutput=True)

    with TileContext(nc) as tc:
        with (
            tc.tile_pool(name="cpool", bufs=1) as cpool,
            tc.tile_pool(name="io", bufs=NT) as io,
            tc.tile_pool(name="lpool", bufs=NT) as lpool,
            tc.tile_pool(name="epool", bufs=3) as epool,
            tc.tile_pool(name="mpool", bufs=2) as mpool,
        ):
            pt, lpvt, envt, acct = {}, {}, {}, {}

            def const_ap(val, tag):
                t = cpool.tile([P, 1], f32, tag=tag)
                nc.vector.memset(t[:], val)
                return t

            c_eps_pos = const_ap(LN_EPS, "c_eps_pos")
            c_eps_neg = const_ap(1.0 + LN_EPS, "c_eps_neg")

            def is_neg(k):
                return k >= NT_BLK

            def emit_dma(k):
                pt[k] = io.tile([P, F], f16, tag="p", name="p_t")
                nc.sync.dma_start(out=pt[k][:], in_=pk[k * P:(k + 1) * P, :])

            def emit_ln(k):
                lpvt[k] = lpool.tile([P, F], f16, tag="lpv", name="lpv_t")
                if is_neg(k):
                    nc.scalar.activation(lpvt[k][:], pt[k][:], AF.Ln,
                                         bias=c_eps_neg[:], scale=-1.0)
                else:
                    nc.scalar.activation(lpvt[k][:], pt[k][:], AF.Ln,
                                         bias=c_eps_pos[:], scale=1.0)

            def emit_env(k):
                envt[k] = epool.tile([P, F], f16, tag="env", name="env_t")
                acct[k] = cpool.tile([P, 4], f32, tag=f"acc{k}", name="acc_t")
                q2, q1 = (N2, N1) if is_neg(k) else (C2, C1)
                nc.vector.affine_mul_reduce(
                    out=envt[k][:], accum_out=acct[k][:, 1:2],
                    in0=pt[k][:], in1=pt[k][:], scale=float(q2), bias=float(q1))

            def emit_m(k):
                q0 = N0 if is_neg(k) else C0
                mt = mpool.tile([P, F], f16, tag="m", name="m_t")
                nc.vector.affine_mul_reduce(
                    out=mt[:], accum_out=acct[k][:, 0:1],
                    in0=envt[k][:], in1=lpvt[k][:], scale=1.0, bias=float(q0))

            for k in range(NT):
                emit_dma(k)
            for k in range(NT):
                emit_ln(k)
            # DVE stream: env0, env1, M0, env2, M1, ... (one-tile stagger so
            # M(k) never waits mid-stream on Act's ln(k))
            emit_env(0)
            for k in range(NT):
                if k + 1 < NT:
                    emit_env(k + 1)
                emit_m(k)
            for k in range(NT):
                nc.sync.dma_start(out=acc_out[:, k:k + 1], in_=acct[k][:, 0:1])

    nc.finalize()
    return nc


def _get_nc():
    if "nc" not in _NC_CACHE:
        _NC_CACHE["nc"] = _build_nc()
    return _NC_CACHE["nc"]


def _run_device(pk_arrs, trace=False, tmpdir=None):
    """pk_arrs: (8, NT*P, F) fp16. Returns (SM, results)."""
    from concourse.bass_utils import run_bass_kernel_spmd

    nc = _get_nc()
    in_maps = [{"pk": pk_arrs[c]} for c in range(N_CORES)]
    res = run_bass_kernel_spmd(
        nc, in_maps, core_ids=list(range(N_CORES)), trace=trace, tmpdir=tmpdir)
    SM = 0.0
    for c in range(N_CORES):
        SM += res.results[c]["acc"].astype(np.float64).sum()
    return SM, res


def _pack_inputs(p_flat, g_flat):
    """Route pred values into padded per-class blocks, fp16, tile-major."""
    pos_v = p_flat[g_flat == 1.0]
    neg_v = p_flat[g_flat != 1.0]
    arrp = np.ones(CAP_BLK, dtype=np.float16)
    arrp[:pos_v.size] = pos_v.astype(np.float16)
    arrn = np.zeros(CAP_BLK, dtype=np.float16)
    arrn[:neg_v.size] = neg_v.astype(np.float16)
    # (cores, P, NT_BLK, F) -> tile-major (cores, NT_BLK, P, F)
    arrp = arrp.reshape(N_CORES, P, NT_BLK, F).swapaxes(1, 2)
    arrn = arrn.reshape(N_CORES, P, NT_BLK, F).swapaxes(1, 2)
    pk = np.concatenate([arrp, arrn], axis=1)          # (cores, NT, P, F)
    return np.ascontiguousarray(pk).reshape(N_CORES, NT * P, F)


def _fallback(pred, gt, mask):
    """Exact numpy mirror of the reference (handles arbitrary inputs)."""
    LOG_EPS = 1e-37
    p = pred[:, 0].astype(np.float64)
    g = gt[:, 0].astype(np.float64)
    m = mask.astype(np.float64)
    positive = g * m
    negative = (1.0 - g) * m
    pos_cnt = positive.sum()
    neg_cnt = min(negative.sum(), np.floor(pos_cnt * NEGATIVE_RATIO))
    loss = ((g - 1.0) * np.log(1.0 - p + LOG_EPS) / np.exp(1.0 - p)
            - g * np.log(p + LOG_EPS) / np.exp(p))
    pos_loss = (loss * positive).sum()
    flat_neg = (loss * negative).ravel()
    k = int(np.ceil(neg_cnt - 1e-12)) if neg_cnt > 0 else 0
    if k >= flat_neg.size:
        neg_sum = flat_neg.sum()
    elif k > 0:
        neg_sum = np.partition(flat_neg, flat_neg.size - k)[flat_neg.size - k:].sum()
    else:
        neg_sum = 0.0
    return np.float32((pos_loss + neg_sum) / (pos_cnt + neg_cnt + EPS))


def kernel(pred, gt, mask):
    pred = np.asarray(pred)
    gt = np.asarray(gt)
    mask = np.asarray(mask)
    if not (mask == 1.0).all() or not ((gt == 0.0) | (gt == 1.0)).all():
        return _fallback(pred, gt, mask)

    g_flat = gt.ravel()
    p_flat = np.ascontiguousarray(pred, dtype=np.float32).ravel()
    n_pos = int(np.count_nonzero(g_flat))
    n_neg = TOTAL - n_pos
    if n_pos > CAP_BLK or n_neg > CAP_BLK:
        return _fallback(pred, gt, mask)

    pos_cnt = float(n_pos)
    neg_raw = float(n_neg)
    neg_count = min(neg_raw, float(np.floor(np.float32(pos_cnt) * np.float32(NEGATIVE_RATIO))))
    if neg_raw > neg_count + 0.5:
        # top-k actually bites; take the exact path
        return _fallback(pred, gt, mask)

    pk = _pack_inputs(p_flat, g_flat)
    SM, _ = _run_device(pk)
    return np.float32(-SM / (pos_cnt + neg_count + EPS))


# revision 5
# speedup vs baseline: 1.7230x; 1.0119x over previous
"""BalanceCrossEntropyLoss on 8 Trainium2 NeuronCores.

Problem shapes (hardcoded): pred (16,1,1024,1024) f32, gt (16,1,1024,1024) f32,
mask (16,1024,1024) f32.  Output: scalar f32.

Math
----
With binary gt and an all-ones mask (verified on host; exact fallback
otherwise), every flattened negative-loss entry at a negative pixel is > 0
and every other entry is exactly 0, so whenever #neg <= floor(3*#pos) the
reference's hard-negative top-k selects *all* negatives and

    numerator = sum(positive_loss) + negative_sum = sum_i loss_i = -SM,
    SM = sum_i ln(v_i + eps)*exp(-v_i),   v_i = p_i if g_i=1 else 1-p_i,
    balance_loss = -SM / (#pos + #neg + 1e-6).

So the device only needs the single reduction SM; the counts come from the
host-side validation pass that is required anyway.

Sharding strategy
-----------------
Shard by class: the host routes each pred value into a "positive" or
"negative" column block (a permutation / bucketing of the input driven by
the binary gt mask -- gt itself never needs to be uploaded) and pads the
blocks to a fixed per-core size with neutral values (p=1 for the positive
block, p=0 for the negative block; each contributes ~4e-8 to SM).  Values
are shipped as fp16 (rel. rounding 2^-11; validated ~1e-3 relative error on
the final scalar vs the 2e-2 budget).

Device kernel (per core, 2 blocks x 3 tiles of [128, 2752] fp16):
    lpv  = Ln(s*p + b)            ScalarE, s=+1,b=eps (pos) / s=-1,b=1+eps (neg)
    env1 = (c2*p + c1)*p          VectorE affine_mul_reduce (2x fp16 mode)
    M    = (env1 + c0)*lpv        VectorE amr, accum_out -> per-partition SM
with (c0,c1,c2) a bias-corrected least-squares quadratic for exp(-t) on
[0,1] (max rel err 1.4e-2, zero mean error against the ln weight; the
negative block uses the mirrored coefficients for exp(t-1)).  No gt tensor,
no matmuls: 4.1MB HBM in per core, 2 DVE ops + 1 Act op per element.
"""

import os
import sys

sys.path.insert(0, "/opt/trn_rl_repo")

import numpy as np
import ml_dtypes

BF16 = ml_dtypes.bfloat16

N_CORES = 8
P = 128
F = 2752                  # tile free dim
NT_BLK = 3                # tiles per class block
NT = 2 * NT_BLK           # 6 tiles per core
KBLK = NT_BLK * F         # 8256 columns per class block per core
CAP_BLK = N_CORES * P * KBLK   # 8454144 element capacity per class
TOTAL = 16 * 1024 * 1024
LN_EPS = 1e-7             # device Ln bias; baseline-validated vs the 1e-37 ref
NEGATIVE_RATIO = 3.0
EPS = 1e-6

# bias-corrected LS quadratic for exp(-t), t in [0,1]  (see module docstring)
C0, C1, C2 = 0.99493479, -0.93054858, 0.30871856
# mirrored coefficients: exp(p-1) = N0 + N1*p + N2*p^2
N0, N1, N2 = C0 + C1 + C2, -C1 - 2.0 * C2, C2

_NC_CACHE = {}


def _patch_act_tables():
    """Restrict Ln/Exp to the combined 'natural_log_exp_and_others' table so
    the act-table-load pass emits one hoisted load instead of per-tile
    switches."""
    import concourse.bacc as bacc_mod
    import concourse.mybir as mybir
    from concourse.hw_specs import get_activation_tables as _real

    if getattr(bacc_mod, "_act_tables_patched", False):
        return

    AF = mybir.ActivationFunctionType

    def _combined(arch):
        out = {}
        for name, funcs in _real(arch).items():
            if name == "natural_log_exp_and_others":
                out[name] = set(funcs)
            else:
                out[name] = set(funcs) - {AF.Ln, AF.Exp}
        return out

    bacc_mod.get_activation_tables = _combined
    bacc_mod._act_tables_patched = True


def _build_nc(debug=False):
    import concourse.bacc as bacc
    import concourse.mybir as mybir
    from concourse.tile import TileContext

    f32 = mybir.dt.float32
    f16 = mybir.dt.bfloat16
    AF = mybir.ActivationFunctionType

    _patch_act_tables()
    nc = bacc.Bacc(None, target_bir_lowering=False, debug=debug)
    # tile-major layout: tile k = rows [128k, 128k+128)
    pk = nc.declare_dram_parameter("pk", [NT * P, F], f16, isOutput=False)
    acc_out = nc.declare_dram_parameter("acc", [P, NT], f32, isOutput=True)

    with TileContext(nc) as tc:
        with (
            tc.tile_pool(name="cpool", bufs=1) as cpool,
            tc.tile_pool(name="io", bufs=NT) as io,
            tc.tile_pool(name="lpool", bufs=NT) as lpool,
            tc.tile_pool(name="epool", bufs=3) as epool,
            tc.tile_pool(name="mpool", bufs=2) as mpool,
        ):
            pt, lpvt, envt, acct = {}, {}, {}, {}

            def const_ap(val, tag):
                t = cpool.tile([P, 1], f32, tag=tag)
                nc.vector.memset(t[:], val)
                return t

            c_eps_pos = const_ap(LN_EPS, "c_eps_pos")
            c_eps_neg = const_ap(1.0 + LN_EPS, "c_eps_neg")

            def is_neg(k):
                return k >= NT_BLK

            def emit_dma(k):
                pt[k] = io.tile([P, F], f16, tag="p", name="p_t")
                nc.sync.dma_start(out=pt[k][:], in_=pk[k * P:(k + 1) * P, :])

            def emit_ln(k):
                lpvt[k] = lpool.tile([P, F], f16, tag="lpv", name="lpv_t")
                if is_neg(k):
                    nc.scalar.activation(lpvt[k][:], pt[k][:], AF.Ln,
                                         bias=c_eps_neg[:], scale=-1.0)
                else:
                    nc.scalar.activation(lpvt[k][:], pt[k][:], AF.Ln,
                                         bias=c_eps_pos[:], scale=1.0)

            def emit_env(k):
                envt[k] = epool.tile([P, F], f16, tag="env", name="env_t")
                acct[k] = cpool.tile([P, 4], f32, tag=f"acc{k}", name="acc_t")
                q2, q1 = (N2, N1) if is_neg(k) else (C2, C1)
                nc.vector.affine_mul_reduce(
                    out=envt[k][:], accum_out=acct[k][:, 1:2],
                    in0=pt[k][:], in1=pt[k][:], scale=float(q2), bias=float(q1))

            def emit_m(k):
                q0 = N0 if is_neg(k) else C0
                mt = mpool.tile([P, F], f16, tag="m", name="m_t")
                nc.vector.affine_mul_reduce(
                    out=mt[:], accum_out=acct[k][:, 0:1],
                    in0=envt[k][:], in1=lpvt[k][:], scale=1.0, bias=float(q0))

            for k in range(NT):
                emit_dma(k)
            for k in range(NT):
                emit_ln(k)
            # DVE stream: env0, env1, M0, env2, M1, ... (one-tile stagger so
            # M(k) never waits mid-stream on Act's ln(k))
            emit_env(0)
            for k in range(NT):
                if k + 1 < NT:
                    emit_env(k + 1)
                emit_m(k)
            for k in range(NT):
                nc.sync.dma_start(out=acc_out[:, k:k + 1], in_=acct[k][:, 0:1])

    nc.finalize()
    return nc


def _get_nc():
    if "nc" not in _NC_CACHE:
        _NC_CACHE["nc"] = _build_nc()
    return _NC_CACHE["nc"]


def _run_device(pk_arrs, trace=False, tmpdir=None):
    """pk_arrs: (8, NT*P, F) fp16. Returns (SM, results)."""
    from concourse.bass_utils import run_bass_kernel_spmd

    nc = _get_nc()
    in_maps = [{"pk": pk_arrs[c]} for c in range(N_CORES)]
    res = run_bass_kernel_spmd(
        nc, in_maps, core_ids=list(range(N_CORES)), trace=trace, tmpdir=tmpdir)
    SM = 0.0
    for c in range(N_CORES):
        SM += res.results[c]["acc"].astype(np.float64).sum()
    return SM, res


def _pack_inputs(p_flat, g_flat):
    """Route pred values into padded per-class blocks, fp16, tile-major."""
    pos_v = p_flat[g_flat == 1.0]
    neg_v = p_flat[g_flat != 1.0]
    arrp = np.ones(CAP_BLK, dtype=BF16)
    arrp[:pos_v.size] = pos_v.astype(BF16)
    arrn = np.zeros(CAP_BLK, dtype=BF16)
    arrn[:neg_v.size] = neg_v.astype(BF16)
    # (cores, P, NT_BLK, F) -> tile-major (cores, NT_BLK, P, F)
    arrp = arrp.reshape(N_CORES, P, NT_BLK, F).swapaxes(1, 2)
    arrn = arrn.reshape(N_CORES, P, NT_BLK, F).swapaxes(1, 2)
    pk = np.concatenate([arrp, arrn], axis=1)          # (cores, NT, P, F)
    return np.ascontiguousarray(pk).reshape(N_CORES, NT * P, F)


def _fallback(pred, gt, mask):
    """Exact numpy mirror of the reference (handles arbitrary inputs)."""
    LOG_EPS = 1e-37
    p = pred[:, 0].astype(np.float64)
    g = gt[:, 0].astype(np.float64)
    m = mask.astype(np.float64)
    positive = g * m
    negative = (1.0 - g) * m
    pos_cnt = positive.sum()
    neg_cnt = min(negative.sum(), np.floor(pos_cnt * NEGATIVE_RATIO))
    loss = ((g - 1.0) * np.log(1.0 - p + LOG_EPS) / np.exp(1.0 - p)
            - g * np.log(p + LOG_EPS) / np.exp(p))
    pos_loss = (loss * positive).sum()
    flat_neg = (loss * negative).ravel()
    k = int(np.ceil(neg_cnt - 1e-12)) if neg_cnt > 0 else 0
    if k >= flat_neg.size:
        neg_sum = flat_neg.sum()
    elif k > 0:
        neg_sum = np.partition(flat_neg, flat_neg.size - k)[flat_neg.size - k:].sum()
    else:
        neg_sum = 0.0
    return np.float32((pos_loss + neg_sum) / (pos_cnt + neg_cnt + EPS))


def kernel(pred, gt, mask):
    pred = np.asarray(pred)
    gt = np.asarray(gt)
    mask = np.asarray(mask)
    if not (mask == 1.0).all() or not ((gt == 0.0) | (gt == 1.0)).all():
        return _fallback(pred, gt, mask)

    g_flat = gt.ravel()
    p_flat = np.ascontiguousarray(pred, dtype=np.float32).ravel()
    n_pos = int(np.count_nonzero(g_flat))
    n_neg = TOTAL - n_pos
    if n_pos > CAP_BLK or n_neg > CAP_BLK:
        return _fallback(pred, gt, mask)

    pos_cnt = float(n_pos)
    neg_raw = float(n_neg)
    neg_count = min(neg_raw, float(np.floor(np.float32(pos_cnt) * np.float32(NEGATIVE_RATIO))))
    if neg_raw > neg_count + 0.5:
        # top-k actually bites; take the exact path
        return _fallback(pred, gt, mask)

    pk = _pack_inputs(p_flat, g_flat)
    SM, _ = _run_device(pk)
    return np.float32(-SM / (pos_cnt + neg_count + EPS))
